# revision 1
# baseline (speedup 1.0000x reference)
"""Gemma3 sliding-window attention layer on 8 Trainium2 NeuronCores.

Sharding: tensor-parallel over heads. Core c computes q-head c and kv-head c//2
(kv heads are duplicated across the 2 cores sharing them), then the o_proj
row-slice for its head. The 8 partial o_proj outputs are summed on the host
(the unshard step for row-sharded o_proj).

Device kernel layout choices:
- hidden is fed pre-transposed (hT [HID, S]) so the qkv matmuls contract over
  the partition dim without any on-device transpose.
- q/k are produced in [d, tok] layout (weights stationary); v in [tok, d]
  (hidden stationary). scoresT [keys, q] = kT.T-free @ qT keeps softmax sums
  and the PV contraction on the partition (keys) axis, where PE ones-matmuls
  do the reductions.
- softmax skips max-subtraction (scores are bounded: q/k are RMS-normed), so
  probs accumulate as plain exp and the 1/sum normalization folds into one
  broadcast matmul + multiply at PV evacuation.
- all matmuls run in float32r (full PE rate at free-dim >= 256).
"""
import os
import sys
import types
import contextlib
import ctypes

import numpy as np

for _p in ("/opt/trn_rl_repo", "/root/.axon_site/_ro/trn_rl_repo"):
    if os.path.isdir(_p) and _p not in sys.path:
        sys.path.insert(0, _p)

from contextlib import ExitStack

import concourse.bass as bass
import concourse.mybir as mybir
import concourse.tile as tile
from concourse import bacc
from concourse.bass_utils import run_bass_kernel_spmd

S = 4096
HID = 2560
NH = 8
NKV = 4
HD = 256
WIN = 1024
ROPE_BASE = 10000.0
EPS = 1e-6
SCALING = HD ** -0.5

NCORES = 8
CH = 256            # tokens per chunk/block
NCH = S // CH       # 16
KT = HID // 128     # 20 hid k-tiles
f32 = mybir.dt.float32
f32r = mybir.dt.float32r
AF = mybir.ActivationFunctionType

_NC = None
_last_results = None


def _install_ntff_shim():
    """antenv.axon_hooks is absent in this image; rebuild it over libaxon so
    run_bass_kernel_spmd(trace=True) can capture NTFF profiles."""
    if "antenv.axon_hooks" in sys.modules:
        return
    so_path = "/opt/axon/libaxon_pjrt.so"
    hook = None
    try:
        lib = ctypes.CDLL(so_path)
        if hasattr(lib, "axon_start_nrt_profile"):
            lib.axon_start_nrt_profile.argtypes = [
                ctypes.POINTER(ctypes.c_int64),
                ctypes.c_size_t,
            ]
            lib.axon_start_nrt_profile.restype = ctypes.c_int64
            lib.axon_stop_nrt_profile.argtypes = [ctypes.c_char_p]
            lib.axon_stop_nrt_profile.restype = ctypes.c_int64

            @contextlib.contextmanager
            def _hook(output_dir, device_ids):
                import jax

                jax.devices()
                if device_ids:
                    ids = (ctypes.c_int64 * len(device_ids))(*device_ids)
                    rc = lib.axon_start_nrt_profile(ids, len(device_ids))
                else:
                    rc = lib.axon_start_nrt_profile(None, 0)
                if rc != 0:
                    raise RuntimeError(f"axon_start_nrt_profile rc={rc}")
                try:
                    yield
                finally:
                    n = lib.axon_stop_nrt_profile(str(output_dir).encode())
                    if n < 0:
                        raise RuntimeError(f"axon_stop_nrt_profile rc={n}")

            hook = _hook
    except OSError:
        pass
    mod = types.ModuleType("antenv.axon_hooks")
    mod.get_axon_ntff_profile_hook = lambda: hook
    mod.set_axon_ntff_profile_hook = lambda h: None
    sys.modules["antenv.axon_hooks"] = mod


def _body(ctx, tc, hT, w, ow, cs, msk, nw, on, on1, outp):
    nc = tc.nc

    const = ctx.enter_context(tc.tile_pool(name="const", bufs=1))
    hpool = ctx.enter_context(tc.tile_pool(name="hT", bufs=2))
    cspool = ctx.enter_context(tc.tile_pool(name="cs", bufs=2))
    qpool = ctx.enter_context(tc.tile_pool(name="qT", bufs=2))
    kvpool = ctx.enter_context(tc.tile_pool(name="kv", bufs=6))
    vpool = ctx.enter_context(tc.tile_pool(name="v", bufs=12))
    tmp = ctx.enter_context(tc.tile_pool(name="tmp", bufs=3))
    sqpool = ctx.enter_context(tc.tile_pool(name="sq", bufs=2))
    small = ctx.enter_context(tc.tile_pool(name="small", bufs=3))
    ppool = ctx.enter_context(tc.tile_pool(name="probs", bufs=2))
    apool = ctx.enter_context(tc.tile_pool(name="attnT", bufs=3))
    opool = ctx.enter_context(tc.tile_pool(name="osb", bufs=2))

    rot = ctx.enter_context(tc.tile_pool(name="rot", bufs=5, space="PSUM"))
    pvp = ctx.enter_context(tc.tile_pool(name="pv", bufs=2, space="PSUM"))
    smp = ctx.enter_context(tc.tile_pool(name="sums", bufs=1, space="PSUM"))

    # resident constants
    w_sb = const.tile([128, KT * 768], f32r)
    nc.sync.dma_start(out=w_sb, in_=w)
    ow_sb = const.tile([128, 2 * HID], f32r)
    nc.sync.dma_start(out=ow_sb, in_=ow)
    msk_sb = const.tile([128, 384], f32)
    nc.sync.dma_start(out=msk_sb, in_=msk)
    nw_sb = const.tile([128, 4], f32)
    nc.sync.dma_start(out=nw_sb, in_=nw)
    ones_sb = const.tile([128, 2], f32r)
    nc.sync.dma_start(out=ones_sb, in_=on)
    ones1_sb = const.tile([1, 128], f32r)
    nc.sync.dma_start(out=ones1_sb, in_=on1)

    kv_tiles = {}
    v_tiles = {}

    for t in range(NCH):
        t0 = t * CH

        hTt = hpool.tile([128, KT * CH], f32r, tag="hTt")
        nc.sync.dma_start(out=hTt, in_=hT[:, t * KT * CH:(t + 1) * KT * CH])
        cst = cspool.tile([128, 2 * CH], f32, tag="cst")
        nc.sync.dma_start(out=cst, in_=cs[:, t * 2 * CH:(t + 1) * 2 * CH])
        cos = cst[:, 0:CH]
        sin = cst[:, CH:2 * CH]

        qTt = qpool.tile([128, 2 * CH], f32r, tag="qTt")
        kvt = kvpool.tile([128, 2 * CH], f32r, tag="kvt")

        # q then k: projection -> rmsnorm -> rope, output [d, tok]
        for (j0, wo, dest) in ((0, 0, qTt), (2, 2, kvt)):
            xps = []
            for j in (j0, j0 + 1):
                ps = rot.tile([128, CH], f32, tag="rot")
                for k in range(KT):
                    nc.tensor.matmul(
                        ps,
                        w_sb[:, k * 768 + j * 128:k * 768 + (j + 1) * 128],
                        hTt[:, k * CH:(k + 1) * CH],
                        start=(k == 0), stop=(k == KT - 1))
                xps.append(ps)
            x0p, x1p = xps
            # sum of squares over head_dim via ones-matmul
            sq0 = sqpool.tile([128, CH], f32r, tag="sq")
            sq1 = sqpool.tile([128, CH], f32r, tag="sq")
            nc.scalar.activation(sq0, x0p, AF.Square)
            nc.scalar.activation(sq1, x1p, AF.Square)
            ssq = rot.tile([1, CH], f32, tag="rot")
            nc.tensor.matmul(ssq, ones_sb[:, 0:1], sq0, start=True, stop=False)
            nc.tensor.matmul(ssq, ones_sb[:, 0:1], sq1, start=False, stop=True)
            t1 = small.tile([1, CH], f32, tag="sm1")
            nc.scalar.activation(t1, ssq, AF.Copy, bias=EPS, scale=1.0 / HD)
            t2 = small.tile([1, CH], f32, tag="sm2")
            nc.vector.reciprocal(t2, t1)
            rstd = small.tile([1, CH], f32r, tag="sm3")
            nc.scalar.activation(rstd, t2, AF.Sqrt)
            rb = rot.tile([128, CH], f32, tag="rot")
            nc.tensor.matmul(rb, ones1_sb, rstd, start=True, stop=True)
            # evacuate x*(1+w) from psum on ACT
            x0 = tmp.tile([128, CH], f32, tag="x")
            x1 = tmp.tile([128, CH], f32, tag="x")
            nc.scalar.activation(x0, x0p, AF.Copy, bias=0.0,
                                 scale=nw_sb[:, wo:wo + 1])
            nc.scalar.activation(x1, x1p, AF.Copy, bias=0.0,
                                 scale=nw_sb[:, wo + 1:wo + 2])
            # rope mix
            a = tmp.tile([128, CH], f32, tag="m")
            nc.vector.tensor_mul(a, x0, cos)
            b = tmp.tile([128, CH], f32, tag="m")
            nc.vector.tensor_mul(b, x1, sin)
            e = tmp.tile([128, CH], f32, tag="m")
            nc.vector.tensor_sub(e, a, b)
            c_ = tmp.tile([128, CH], f32, tag="m")
            nc.vector.tensor_mul(c_, x1, cos)
            d = tmp.tile([128, CH], f32, tag="m")
            nc.vector.tensor_mul(d, x0, sin)
            f = tmp.tile([128, CH], f32, tag="m")
            nc.vector.tensor_add(f, c_, d)
            nc.vector.tensor_mul(dest[:, 0:CH], e, rb)
            nc.vector.tensor_mul(dest[:, CH:2 * CH], f, rb)
        kv_tiles[t] = kvt

        # v projection, natural [tok, d] layout
        for st in range(2):
            vps = rot.tile([128, HD], f32, tag="rot")
            for k in range(KT):
                nc.tensor.matmul(
                    vps,
                    hTt[:, k * CH + st * 128:k * CH + st * 128 + 128],
                    w_sb[:, k * 768 + 512:(k + 1) * 768],
                    start=(k == 0), stop=(k == KT - 1))
            vt = vpool.tile([128, HD], f32r, tag="v")
            nc.scalar.activation(vt, vps, AF.Copy, bias=0.0, scale=1.0)
            v_tiles[2 * t + st] = vt

        # attention for the 256 queries of this block
        pv0 = pvp.tile([128, CH], f32, tag="pv")
        pv1 = pvp.tile([128, CH], f32, tag="pv")
        sums = smp.tile([1, CH], f32, tag="sums")
        kts = list(range(max(0, 2 * t - 8), 2 * t + 2))
        for i, kt in enumerate(kts):
            ct, sb = kt // 2, kt % 2
            kvsrc = kv_tiles[ct]
            sc = rot.tile([128, CH], f32, tag="rot")
            for h in range(2):
                nc.tensor.matmul(
                    sc,
                    kvsrc[:, h * CH + sb * 128:h * CH + sb * 128 + 128],
                    qTt[:, h * CH:(h + 1) * CH],
                    start=(h == 0), stop=(h == 1))
            pr = ppool.tile([128, CH], f32r, tag="pr")
            nc.scalar.activation(pr, sc, AF.Exp, bias=0.0, scale=SCALING)
            for sidx, qt in enumerate((2 * t, 2 * t + 1)):
                sl = slice(sidx * 128, (sidx + 1) * 128)
                if kt == qt:
                    m = msk_sb[:, 256:384]
                elif kt > qt or kt < qt - 8:
                    m = msk_sb[:, 128:256]
                elif kt == qt - 8:
                    m = msk_sb[:, 0:128]
                else:
                    m = None
                if m is not None:
                    nc.vector.tensor_mul(pr[:, sl], pr[:, sl], m)
            first, last = (i == 0), (i == len(kts) - 1)
            nc.tensor.matmul(sums, ones_sb[:, 0:1], pr,
                             start=first, stop=last)
            vt = v_tiles[kt]
            nc.tensor.matmul(pv0, vt[:, 0:128], pr, start=first, stop=last)
            nc.tensor.matmul(pv1, vt[:, 128:256], pr, start=first, stop=last)

        inv = small.tile([1, CH], f32r, tag="sm4")
        nc.vector.reciprocal(inv, sums)
        ib = rot.tile([128, CH], f32, tag="rot")
        nc.tensor.matmul(ib, ones1_sb, inv, start=True, stop=True)
        ibs = tmp.tile([128, CH], f32, tag="ibs")
        nc.scalar.activation(ibs, ib, AF.Copy, bias=0.0, scale=1.0)
        at0 = apool.tile([128, CH], f32r, tag="at")
        at1 = apool.tile([128, CH], f32r, tag="at")
        nc.vector.tensor_mul(at0, pv0, ibs)
        nc.vector.tensor_mul(at1, pv1, ibs)

        # o_proj row-slice: partial [256 tok, HID]
        for st in range(2):
            ob = opool.tile([128, HID], f32, tag="ob")
            for hc in range(HID // 512):
                op = rot.tile([128, 512], f32, tag="rot")
                nc.tensor.matmul(op, at0[:, st * 128:(st + 1) * 128],
                                 ow_sb[:, hc * 512:(hc + 1) * 512],
                                 start=True, stop=False)
                nc.tensor.matmul(op, at1[:, st * 128:(st + 1) * 128],
                                 ow_sb[:, HID + hc * 512:HID + (hc + 1) * 512],
                                 start=False, stop=True)
                nc.vector.tensor_copy(ob[:, hc * 512:(hc + 1) * 512], op)
            nc.sync.dma_start(
                out=outp[t0 + st * 128:t0 + (st + 1) * 128, :], in_=ob)


def _build():
    nc = bacc.Bacc("TRN2", target_bir_lowering=False, debug=False,
                   num_devices=NCORES)
    hT = nc.dram_tensor("hT", [128, KT * S], f32r, kind="ExternalInput").ap()
    w = nc.dram_tensor("w", [128, KT * 768], f32r, kind="ExternalInput").ap()
    ow = nc.dram_tensor("ow", [128, 2 * HID], f32r, kind="ExternalInput").ap()
    cs = nc.dram_tensor("cs", [128, NCH * 2 * CH], f32, kind="ExternalInput").ap()
    msk = nc.dram_tensor("msk", [128, 384], f32, kind="ExternalInput").ap()
    nw = nc.dram_tensor("nw", [128, 4], f32, kind="ExternalInput").ap()
    on = nc.dram_tensor("on", [128, 2], f32r, kind="ExternalInput").ap()
    on1 = nc.dram_tensor("on1", [1, 128], f32r, kind="ExternalInput").ap()
    outp = nc.dram_tensor("outp", [S, HID], f32, kind="ExternalOutput").ap()
    with tile.TileContext(nc) as tc, ExitStack() as ctx:
        with nc.allow_low_precision(reason="float32r matmul pipeline"):
            _body(ctx, tc, hT, w, ow, cs, msk, nw, on, on1, outp)
    nc.compile()
    return nc


def _get_nc():
    global _NC
    if _NC is None:
        _NC = _build()
    return _NC


def kernel(positions, hidden_states, qkv_w, o_w, q_norm_w, k_norm_w):
    global _last_results
    _install_ntff_shim()

    positions = np.asarray(positions)
    hidden_states = np.asarray(hidden_states, dtype=np.float32)
    qkv_w = np.asarray(qkv_w, dtype=np.float32)
    o_w = np.asarray(o_w, dtype=np.float32)
    q_norm_w = np.asarray(q_norm_w, dtype=np.float32)
    k_norm_w = np.asarray(k_norm_w, dtype=np.float32)
    assert np.array_equal(positions.astype(np.int64), np.arange(S)), \
        "kernel assumes contiguous arange positions (banded sliding window)"

    hT0 = hidden_states.T  # [HID, S]
    hT = np.ascontiguousarray(
        hT0.reshape(KT, 128, NCH, CH).transpose(1, 2, 0, 3).reshape(128, KT * S))

    inv_freq = 1.0 / (ROPE_BASE ** (np.arange(0, HD, 2, dtype=np.float32) / HD))
    freqs = positions.astype(np.float32)[:, None] * inv_freq[None, :]  # [S,128]
    cos_t = np.ascontiguousarray(np.cos(freqs).T.astype(np.float32))
    sin_t = np.ascontiguousarray(np.sin(freqs).T.astype(np.float32))
    csb = np.stack([cos_t.reshape(128, NCH, CH), sin_t.reshape(128, NCH, CH)],
                   axis=2)  # [128, NCH, 2, CH]
    cs = np.ascontiguousarray(csb.reshape(128, NCH * 2 * CH))

    kl = np.arange(128)[:, None]
    ql = np.arange(128)[None, :]
    edge = (kl > ql).astype(np.float32)
    diag = (kl <= ql).astype(np.float32)
    zero = np.zeros((128, 128), np.float32)
    msk = np.concatenate([edge, zero, diag], axis=1)  # [128, 384]

    nwq = 1.0 + q_norm_w
    nwk = 1.0 + k_norm_w
    nw = np.stack([nwq[:128], nwq[128:], nwk[:128], nwk[128:]], axis=1)
    nw = np.ascontiguousarray(nw.astype(np.float32))  # [128, 4]

    on = np.ones((128, 2), np.float32)
    on1 = np.ones((1, 128), np.float32)

    in_maps = []
    for c in range(NCORES):
        g = c // 2
        wq = qkv_w[:, c * HD:(c + 1) * HD]
        wk = qkv_w[:, NH * HD + g * HD:NH * HD + (g + 1) * HD]
        wv = qkv_w[:, (NH + NKV) * HD + g * HD:(NH + NKV) * HD + (g + 1) * HD]
        wslice = np.concatenate([wq, wk, wv], axis=1).astype(np.float32)
        wslice = np.ascontiguousarray(
            wslice.reshape(KT, 128, 768).transpose(1, 0, 2).reshape(128, KT * 768))
        owslice = o_w[c * HD:(c + 1) * HD, :].astype(np.float32)
        owslice = np.ascontiguousarray(
            owslice.reshape(2, 128, HID).transpose(1, 0, 2).reshape(128, 2 * HID))
        in_maps.append({
            "hT": hT, "w": wslice, "ow": owslice, "cs": cs, "msk": msk,
            "nw": nw, "on": on, "on1": on1,
        })

    nc = _get_nc()
    res = run_bass_kernel_spmd(nc, in_maps, list(range(NCORES)))
    _last_results = res

    out = res.results[0]["outp"].astype(np.float32).copy()
    for c in range(1, NCORES):
        out += res.results[c]["outp"]
    return out



# revision 10
# speedup vs baseline: 1.6375x; 1.6375x over previous
"""Gemma3 sliding-window attention layer on 8 Trainium2 NeuronCores.

Sharding: tensor-parallel over heads. Core c computes q-head c and kv-head c//2
(kv heads are duplicated across the 2 cores sharing them), then the o_proj
row-slice for its head. The 8 partial o_proj outputs are summed on the host.

v2 design notes (vs the fp32r baseline):
- All matmul operands are bf16 (except tiny f32r broadcast helpers): FWL halves
  LDWEIGHTS, DMA bytes halve, and the rel-err budget (2e-2) has 80x margin.
- 512-token chunks with N=512 matmul free dims: LDWEIGHTS fully hidden under
  the 213ns streams, fewer instructions.
- RMSNorm (1+w) scale is folded into the qkv weights on the host; the ssq
  ones-matmul uses 1/(1+w)^2 stationary columns to recover sum(x^2).
- Attention score/PV matmuls are trimmed to the sliding window (per key tile,
  only the query blocks that can see it).
- softmax sums accumulate per-element (has_written) into one PSUM bank; the
  1/sum reciprocal (fast approx) and the o_proj of chunk t-1 are pipelined
  into chunk t's projection phase so the PE never waits on them.
- ACT runs only Exp/Sqrt/Copy; squares and evacuations run on DVE.
"""
import os
import sys
import types
import contextlib
import ctypes

import numpy as np
import ml_dtypes

for _p in ("/opt/trn_rl_repo", "/root/.axon_site/_ro/trn_rl_repo"):
    if os.path.isdir(_p) and _p not in sys.path:
        sys.path.insert(0, _p)

from contextlib import ExitStack

import concourse.bass as bass
import concourse.mybir as mybir
import concourse.tile as tile
from concourse import bacc
from concourse.bass_utils import run_bass_kernel_spmd

S = 4096
HID = 2560
NH = 8
NKV = 4
HD = 256
WIN = 1024
ROPE_BASE = 10000.0
EPS = 1e-6
SCALING = HD ** -0.5

NCORES = 8
CH = 512            # tokens per chunk
NCH = S // CH       # 8
KT = HID // 128     # 20 hid k-tiles
QB = CH // 128      # 4 query 128-blocks per chunk
f32 = mybir.dt.float32
f32r = mybir.dt.float32r
bf16 = mybir.dt.bfloat16
AF = mybir.ActivationFunctionType
BF16 = ml_dtypes.bfloat16

_NC = None
_last_results = None


def _install_ntff_shim():
    """antenv.axon_hooks is absent in this image; rebuild it over libaxon so
    run_bass_kernel_spmd(trace=True) can capture NTFF profiles."""
    if "antenv.axon_hooks" in sys.modules:
        return
    so_path = "/opt/axon/libaxon_pjrt.so"
    hook = None
    try:
        lib = ctypes.CDLL(so_path)
        if hasattr(lib, "axon_start_nrt_profile"):
            lib.axon_start_nrt_profile.argtypes = [
                ctypes.POINTER(ctypes.c_int64),
                ctypes.c_size_t,
            ]
            lib.axon_start_nrt_profile.restype = ctypes.c_int64
            lib.axon_stop_nrt_profile.argtypes = [ctypes.c_char_p]
            lib.axon_stop_nrt_profile.restype = ctypes.c_int64

            @contextlib.contextmanager
            def _hook(output_dir, device_ids):
                import jax

                jax.devices()
                if device_ids:
                    ids = (ctypes.c_int64 * len(device_ids))(*device_ids)
                    rc = lib.axon_start_nrt_profile(ids, len(device_ids))
                else:
                    rc = lib.axon_start_nrt_profile(None, 0)
                if rc != 0:
                    raise RuntimeError(f"axon_start_nrt_profile rc={rc}")
                try:
                    yield
                finally:
                    n = lib.axon_stop_nrt_profile(str(output_dir).encode())
                    if n < 0:
                        raise RuntimeError(f"axon_stop_nrt_profile rc={n}")

            hook = _hook
    except OSError:
        pass
    mod = types.ModuleType("antenv.axon_hooks")
    mod.get_axon_ntff_profile_hook = lambda: hook
    mod.set_axon_ntff_profile_hook = lambda h: None
    sys.modules["antenv.axon_hooks"] = mod


def _qrange(kappa, t):
    # relative query 128-block range in chunk t that can see key tile kappa
    qlo = max(kappa - 4 * t, 0)
    qhi = min(kappa + 8 - 4 * t, QB - 1)
    return qlo, qhi


def _body(ctx, tc, hT, w, ow, cs, msk, invsq, on1, onecol, outp):
    nc = tc.nc

    const = ctx.enter_context(tc.tile_pool(name="const", bufs=1))
    hpool = ctx.enter_context(tc.tile_pool(name="hT", bufs=2))
    cspool = ctx.enter_context(tc.tile_pool(name="cs", bufs=2))
    sqpool = ctx.enter_context(tc.tile_pool(name="sq", bufs=4))
    qpool = ctx.enter_context(tc.tile_pool(name="qT", bufs=2))
    kvpool = ctx.enter_context(tc.tile_pool(name="kv", bufs=3))
    vpool = ctx.enter_context(tc.tile_pool(name="v", bufs=12))
    prpool = ctx.enter_context(tc.tile_pool(name="pr", bufs=4))
    atpool = ctx.enter_context(tc.tile_pool(name="at", bufs=4))
    ibspool = ctx.enter_context(tc.tile_pool(name="ibs", bufs=2))
    small = ctx.enter_context(tc.tile_pool(name="small", bufs=2))
    opool = ctx.enter_context(tc.tile_pool(name="osb", bufs=3))

    # PSUM: exactly 8 banks
    xp = ctx.enter_context(tc.tile_pool(name="xp", bufs=4, space="PSUM"))
    pvp = ctx.enter_context(tc.tile_pool(name="pv", bufs=2, space="PSUM"))
    rbp = ctx.enter_context(tc.tile_pool(name="rb", bufs=2, space="PSUM"))

    # resident constants (weight DMA split so the first matmuls start early)
    w_sb = const.tile([128, KT * 768], bf16)
    for piece in range(4):
        nc.sync.dma_start(
            out=w_sb[:, piece * 5 * 768:(piece + 1) * 5 * 768],
            in_=w[:, piece * 5 * 768:(piece + 1) * 5 * 768])
    ow_sb = const.tile([128, 2 * HID], bf16)
    nc.sync.dma_start(out=ow_sb, in_=ow)
    msk_sb = const.tile([128, 256], bf16)
    nc.sync.dma_start(out=msk_sb, in_=msk)
    invsq_sb = const.tile([128, 4], f32r)
    nc.sync.dma_start(out=invsq_sb, in_=invsq)
    on1_sb = const.tile([1, 128], f32r)
    nc.sync.dma_start(out=on1_sb, in_=on1)
    onecol_sb = const.tile([128, 1], bf16)
    nc.sync.dma_start(out=onecol_sb, in_=onecol)

    kv_tiles = {}
    v_tiles = {}
    # carried across chunk iterations (chunk t-1 state)
    carry = {}

    def emit_ib(tp):
        # ib broadcast of 1/sums for chunk tp, then at = pv * ibs on DVE
        c = carry
        ib = rbp.tile([128, CH], f32, tag="rbt")
        nc.tensor.matmul(ib, on1_sb, c["invr"], start=True, stop=True)
        ibs = ibspool.tile([128, CH], f32, tag="ibs")
        nc.vector.tensor_copy(ibs, ib)
        at0 = atpool.tile([128, CH], bf16, tag="at")
        at1 = atpool.tile([128, CH], bf16, tag="at")
        nc.vector.tensor_mul(at0, c["pv0"], ibs)
        nc.vector.tensor_mul(at1, c["pv1"], ibs)
        c["at"] = (at0, at1)

    def emit_oproj(tp, st_list):
        at0, at1 = carry["at"]
        for st in st_list:
            ob = opool.tile([128, HID], bf16, tag="ob")
            for hc in range(HID // 512):
                op = xp.tile([128, 512], f32, tag="mm")
                nc.tensor.matmul(op, at0[:, st * 128:(st + 1) * 128],
                                 ow_sb[:, hc * 512:(hc + 1) * 512],
                                 start=True, stop=False)
                nc.tensor.matmul(op, at1[:, st * 128:(st + 1) * 128],
                                 ow_sb[:, HID + hc * 512:HID + (hc + 1) * 512],
                                 start=False, stop=True)
                nc.vector.tensor_copy(ob[:, hc * 512:(hc + 1) * 512], op)
            nc.sync.dma_start(
                out=outp[tp * CH + st * 128:tp * CH + (st + 1) * 128, :],
                in_=ob)

    def rstd_chain(ssqt, row, tag):
        t1 = small.tile([1, CH], f32, tag=f"t1{tag}")
        nc.scalar.activation(t1, ssqt[row:row + 1, :], AF.Copy,
                             bias=EPS, scale=1.0 / HD)
        r0 = small.tile([1, CH], f32, tag=f"r0{tag}")
        nc.vector.reciprocal_approx_fast(out=r0, in_=t1)
        rstd = small.tile([1, CH], f32r, tag=f"rs{tag}")
        nc.scalar.activation(rstd, r0, AF.Sqrt)
        return rstd

    for t in range(NCH):
        # ---- input DMA (4 pieces so compute starts early) ----
        hTt = hpool.tile([128, KT * CH], bf16, tag="hTt")
        for piece in range(4):
            lo = t * KT * CH + piece * 5 * CH
            nc.sync.dma_start(
                out=hTt[:, piece * 5 * CH:(piece + 1) * 5 * CH],
                in_=hT[:, lo:lo + 5 * CH])
        cst = cspool.tile([128, 2 * CH], f32, tag="cst")
        nc.sync.dma_start(out=cst, in_=cs[:, t * 2 * CH:(t + 1) * 2 * CH])
        cos = cst[:, 0:CH]
        sin = cst[:, CH:2 * CH]

        # ---- q projection (j=0,1) ----
        qx = []
        for j in (0, 1):
            ps = xp.tile([128, CH], f32, tag="mm")
            for k in range(KT):
                nc.tensor.matmul(
                    ps, w_sb[:, k * 768 + j * 128:k * 768 + (j + 1) * 128],
                    hTt[:, k * CH:(k + 1) * CH],
                    start=(k == 0), stop=(k == KT - 1))
            qx.append(ps)

        # ib + at for chunk t-1 (hides the softmax reciprocal latency)
        if t > 0:
            emit_ib(t - 1)

        # squares for q on ACT (DVE cannot read two PSUM operands)
        sq_q = []
        for j in (0, 1):
            sq = sqpool.tile([128, CH], f32r, tag="sq")
            nc.scalar.activation(sq, qx[j], AF.Square)
            sq_q.append(sq)

        # ---- k projection j=0 ----
        kx = []
        ps = xp.tile([128, CH], f32, tag="mm")
        for k in range(KT):
            nc.tensor.matmul(
                ps, w_sb[:, k * 768 + 256:k * 768 + 384],
                hTt[:, k * CH:(k + 1) * CH],
                start=(k == 0), stop=(k == KT - 1))
        kx.append(ps)

        # ssq for q
        ssq_q = rbp.tile([1, CH], f32, tag="rbt")
        nc.tensor.matmul(ssq_q, invsq_sb[:, 0:1], sq_q[0],
                         start=True, stop=False)
        nc.tensor.matmul(ssq_q, invsq_sb[:, 1:2], sq_q[1],
                         start=False, stop=True)

        # ---- k projection j=1 ----
        ps = xp.tile([128, CH], f32, tag="mm")
        for k in range(KT):
            nc.tensor.matmul(
                ps, w_sb[:, k * 768 + 384:k * 768 + 512],
                hTt[:, k * CH:(k + 1) * CH],
                start=(k == 0), stop=(k == KT - 1))
        kx.append(ps)

        rstd_q = rstd_chain(ssq_q, 0, "q")
        rb_q = rbp.tile([128, CH], f32, tag="rbt")
        nc.tensor.matmul(rb_q, on1_sb, rstd_q, start=True, stop=True)

        # rope for q on DVE -> qTt [d, tok] bf16
        qTt = qpool.tile([128, 2 * CH], bf16, tag="qTt")
        a = sqpool.tile([128, CH], f32, tag="rm")
        b = sqpool.tile([128, CH], f32, tag="rm")
        nc.vector.tensor_mul(a, qx[0], cos)
        nc.vector.tensor_mul(b, qx[1], sin)
        e = sqpool.tile([128, CH], f32, tag="rm")
        nc.vector.tensor_sub(e, a, b)
        nc.vector.tensor_mul(a, qx[1], cos)
        nc.vector.tensor_mul(b, qx[0], sin)
        f_ = sqpool.tile([128, CH], f32, tag="rm")
        nc.vector.tensor_add(f_, a, b)
        nc.vector.tensor_mul(qTt[:, 0:CH], e, rb_q)
        nc.vector.tensor_mul(qTt[:, CH:2 * CH], f_, rb_q)

        # squares for k
        sq_k = []
        for j in (0, 1):
            sq = sqpool.tile([128, CH], f32r, tag="sq")
            nc.scalar.activation(sq, kx[j], AF.Square)
            sq_k.append(sq)

        # o_proj for chunk t-1, first half
        if t > 0:
            emit_oproj(t - 1, (0, 1))

        # ssq for k (row 1, bank already cleared by row 0's start)
        ssq_k = rbp.tile([1, CH], f32, tag="rbt")
        nc.tensor.matmul(ssq_k, invsq_sb[:, 2:3], sq_k[0],
                         start=True, stop=False)
        nc.tensor.matmul(ssq_k, invsq_sb[:, 3:4], sq_k[1],
                         start=False, stop=True)
        rstd_k = rstd_chain(ssq_k, 0, "k")
        rb_k = rbp.tile([128, CH], f32, tag="rbt")
        nc.tensor.matmul(rb_k, on1_sb, rstd_k, start=True, stop=True)

        if t > 0:
            emit_oproj(t - 1, (2, 3))

        # rope for k -> kvt [d, tok] bf16
        kvt = kvpool.tile([128, 2 * CH], bf16, tag="kvt")
        a2 = sqpool.tile([128, CH], f32, tag="rm")
        b2 = sqpool.tile([128, CH], f32, tag="rm")
        nc.vector.tensor_mul(a2, kx[0], cos)
        nc.vector.tensor_mul(b2, kx[1], sin)
        e2 = sqpool.tile([128, CH], f32, tag="rm")
        nc.vector.tensor_sub(e2, a2, b2)
        nc.vector.tensor_mul(a2, kx[1], cos)
        nc.vector.tensor_mul(b2, kx[0], sin)
        f2 = sqpool.tile([128, CH], f32, tag="rm")
        nc.vector.tensor_add(f2, a2, b2)
        nc.vector.tensor_mul(kvt[:, 0:CH], e2, rb_k)
        nc.vector.tensor_mul(kvt[:, CH:2 * CH], f2, rb_k)
        kv_tiles[t] = kvt

        # ---- v projection, [tok, d] layout ----
        for st in range(QB):
            vps = xp.tile([128, HD], f32, tag="mm")
            for k in range(KT):
                nc.tensor.matmul(
                    vps, hTt[:, k * CH + st * 128:k * CH + st * 128 + 128],
                    w_sb[:, k * 768 + 512:(k + 1) * 768],
                    start=(k == 0), stop=(k == KT - 1))
            vt = vpool.tile([128, HD], bf16, tag="v")
            nc.vector.tensor_copy(vt, vps)
            v_tiles[QB * t + st] = vt

        # ---- attention for the 512 queries of this chunk ----
        pv0 = pvp.tile([128, CH], f32, tag="pv")
        pv1 = pvp.tile([128, CH], f32, tag="pv")
        sums = rbp.tile([1, CH], f32, tag="rbt")
        kts = list(range(max(0, 4 * t - 8), 4 * t + 4))

        def sc_mm(kappa):
            qlo, qhi = _qrange(kappa, t)
            cols = slice(qlo * 128, (qhi + 1) * 128)
            ct, sb = kappa // QB, kappa % QB
            kvsrc = kv_tiles[ct]
            sc = xp.tile([128, CH], f32, tag="mm")
            for h in range(2):
                nc.tensor.matmul(
                    sc[:, cols],
                    kvsrc[:, h * CH + sb * 128:h * CH + sb * 128 + 128],
                    qTt[:, h * CH + qlo * 128:h * CH + (qhi + 1) * 128],
                    start=(h == 0), stop=(h == 1))
            return sc

        def exp_mask(kappa, sc):
            qlo, qhi = _qrange(kappa, t)
            cols = slice(qlo * 128, (qhi + 1) * 128)
            pr = prpool.tile([128, CH], bf16, tag="pr")
            nc.scalar.activation(pr[:, cols], sc[:, cols], AF.Exp,
                                 bias=0.0, scale=SCALING)
            # diag mask where query block == kappa (first block of range)
            if kappa - 4 * t == qlo:
                dsl = slice(qlo * 128, (qlo + 1) * 128)
                nc.vector.tensor_mul(pr[:, dsl], pr[:, dsl],
                                     msk_sb[:, 128:256])
            # edge mask where query block == kappa + 8 (last block of range)
            if kappa + 8 - 4 * t == qhi:
                esl = slice(qhi * 128, (qhi + 1) * 128)
                nc.vector.tensor_mul(pr[:, esl], pr[:, esl],
                                     msk_sb[:, 0:128])
            return pr

        def sums_pv(kappa, pr, first, last):
            qlo, qhi = _qrange(kappa, t)
            cols = slice(qlo * 128, (qhi + 1) * 128)
            nc.tensor.matmul(sums[:, cols], onecol_sb, pr[:, cols],
                             start=first, stop=last, skip_group_check=True)
            vt = v_tiles[kappa]
            nc.tensor.matmul(pv0[:, cols], vt[:, 0:128], pr[:, cols],
                             start=first, stop=last, skip_group_check=True)
            nc.tensor.matmul(pv1[:, cols], vt[:, 128:256], pr[:, cols],
                             start=first, stop=last, skip_group_check=True)

        sc_prev = sc_mm(kts[0])
        pr_prev = exp_mask(kts[0], sc_prev)
        for i, kappa in enumerate(kts[1:], start=1):
            sc = sc_mm(kappa)
            sums_pv(kts[i - 1], pr_prev, first=(i == 1), last=False)
            pr_prev = exp_mask(kappa, sc)
        sums_pv(kts[-1], pr_prev, first=(len(kts) == 1), last=True)

        # 1/sums (fast approx) -> f32r for next chunk's ib matmul
        inv0 = small.tile([1, CH], f32, tag="inv0")
        nc.vector.reciprocal_approx_fast(out=inv0, in_=sums)
        invr = small.tile([1, CH], f32r, tag="invr")
        nc.vector.tensor_copy(invr, inv0)
        carry["invr"] = invr
        carry["pv0"] = pv0
        carry["pv1"] = pv1

    # tail: chunk NCH-1 normalization + o_proj
    emit_ib(NCH - 1)
    emit_oproj(NCH - 1, (0, 1, 2, 3))


def _build():
    nc = bacc.Bacc("TRN2", target_bir_lowering=False, debug=False,
                   num_devices=NCORES)
    hT = nc.dram_tensor("hT", [128, KT * S], bf16, kind="ExternalInput").ap()
    w = nc.dram_tensor("w", [128, KT * 768], bf16, kind="ExternalInput").ap()
    ow = nc.dram_tensor("ow", [128, 2 * HID], bf16, kind="ExternalInput").ap()
    cs = nc.dram_tensor("cs", [128, NCH * 2 * CH], f32, kind="ExternalInput").ap()
    msk = nc.dram_tensor("msk", [128, 256], bf16, kind="ExternalInput").ap()
    invsq = nc.dram_tensor("invsq", [128, 4], f32r, kind="ExternalInput").ap()
    on1 = nc.dram_tensor("on1", [1, 128], f32r, kind="ExternalInput").ap()
    onecol = nc.dram_tensor("onecol", [128, 1], bf16, kind="ExternalInput").ap()
    outp = nc.dram_tensor("outp", [S, HID], bf16, kind="ExternalOutput").ap()
    with tile.TileContext(nc) as tc, ExitStack() as ctx:
        with nc.allow_low_precision(reason="bf16 matmul pipeline"):
            _body(ctx, tc, hT, w, ow, cs, msk, invsq, on1, onecol, outp)
    nc.compile()
    return nc


def _get_nc():
    global _NC
    if _NC is None:
        _NC = _build()
    return _NC


def kernel(positions, hidden_states, qkv_w, o_w, q_norm_w, k_norm_w):
    global _last_results
    _install_ntff_shim()

    positions = np.asarray(positions)
    hidden_states = np.asarray(hidden_states, dtype=np.float32)
    qkv_w = np.asarray(qkv_w, dtype=np.float32)
    o_w = np.asarray(o_w, dtype=np.float32)
    q_norm_w = np.asarray(q_norm_w, dtype=np.float32)
    k_norm_w = np.asarray(k_norm_w, dtype=np.float32)
    assert np.array_equal(positions.astype(np.int64), np.arange(S)), \
        "kernel assumes contiguous arange positions (banded sliding window)"

    hT0 = hidden_states.T  # [HID, S]
    hT = np.ascontiguousarray(
        hT0.reshape(KT, 128, NCH, CH).transpose(1, 2, 0, 3)
        .reshape(128, KT * S)).astype(BF16)

    inv_freq = 1.0 / (ROPE_BASE ** (np.arange(0, HD, 2, dtype=np.float32) / HD))
    freqs = positions.astype(np.float32)[:, None] * inv_freq[None, :]  # [S,128]
    cos_t = np.cos(freqs).T.astype(np.float32)
    sin_t = np.sin(freqs).T.astype(np.float32)
    csb = np.stack([cos_t.reshape(128, NCH, CH), sin_t.reshape(128, NCH, CH)],
                   axis=2)  # [128, NCH, 2, CH]
    cs = np.ascontiguousarray(csb.reshape(128, NCH * 2 * CH))

    kl = np.arange(128)[:, None]
    ql = np.arange(128)[None, :]
    edge = (kl > ql).astype(BF16)
    diag = (kl <= ql).astype(BF16)
    msk = np.ascontiguousarray(np.concatenate([edge, diag], axis=1))

    nwq = 1.0 + q_norm_w
    nwk = 1.0 + k_norm_w
    iq = 1.0 / (nwq * nwq)
    ik = 1.0 / (nwk * nwk)
    invsq = np.ascontiguousarray(
        np.stack([iq[:128], iq[128:], ik[:128], ik[128:]], axis=1)
        .astype(np.float32))

    on1 = np.ones((1, 128), np.float32)
    onecol = np.ones((128, 1), BF16)

    in_maps = []
    for c in range(NCORES):
        g = c // 2
        wq = qkv_w[:, c * HD:(c + 1) * HD] * nwq[None, :]
        wk = qkv_w[:, NH * HD + g * HD:NH * HD + (g + 1) * HD] * nwk[None, :]
        wv = qkv_w[:, (NH + NKV) * HD + g * HD:(NH + NKV) * HD + (g + 1) * HD]
        wslice = np.concatenate([wq, wk, wv], axis=1).astype(np.float32)
        wslice = np.ascontiguousarray(
            wslice.reshape(KT, 128, 768).transpose(1, 0, 2)
            .reshape(128, KT * 768)).astype(BF16)
        owslice = o_w[c * HD:(c + 1) * HD, :].astype(np.float32)
        owslice = np.ascontiguousarray(
            owslice.reshape(2, 128, HID).transpose(1, 0, 2)
            .reshape(128, 2 * HID)).astype(BF16)
        in_maps.append({
            "hT": hT, "w": wslice, "ow": owslice, "cs": cs, "msk": msk,
            "invsq": invsq, "on1": on1, "onecol": onecol,
        })

    nc = _get_nc()
    res = run_bass_kernel_spmd(nc, in_maps, list(range(NCORES)))
    _last_results = res

    out = res.results[0]["outp"].astype(np.float32)
    for c in range(1, NCORES):
        out = out + res.results[c]["outp"].astype(np.float32)
    return out
